# revision 5
# baseline (speedup 1.0000x reference)
"""Causal attention (QKV proj + per-head RMSNorm + causal softmax + out proj)
on 8 Trainium2 NeuronCores.

Sharding: batch (4) x head-group (2 groups of 8 heads) -> 8 cores.
Each core computes, for its (batch b, head group g):
  Q^T, K^T in [d_head-on-partition, seq] layout (transposed via host-side x^T),
  per-head RMSNorm fused via ones-block matmul + broadcast matmul,
  S^T = K^T.T @ Q^T per (k-chunk 128, q-block 512), exp on ScalarE,
  causal masking via precomputed 0/1 prefix tiles (diagonal chunks only;
  strictly-above-diagonal chunks are skipped),
  z^T = sum_k V[k,:]^aug P^T[k,:]  (V augmented with a ones column -> row 64
  of the PSUM result is the softmax denominator r),
  z normalized by 1/r (partition_broadcast + fused multiply on eviction),
  out^T = W_o_g^T z^T accumulated per q-block, DMA'd out.
Host sums the two head-group partials per batch, transposes, adds b_o.
"""
import sys
import numpy as np

sys.path.insert(0, "/opt/trn_rl_repo")

import concourse.bass as bass  # noqa: E402
import concourse.bacc as bacc  # noqa: E402
import concourse.tile as tile  # noqa: E402
from concourse import mybir  # noqa: E402
from concourse.bass_utils import run_bass_kernel_spmd  # noqa: E402
from contextlib import ExitStack  # noqa: E402

F32 = mybir.dt.float32
AF = mybir.ActivationFunctionType
ALU = mybir.AluOpType

D = 1024
S = 2048
N_HEAD = 16
DH = 64
HEADS_PER_CORE = 8  # 2 head-groups of 8 heads
QKCOLS = 2 * HEADS_PER_CORE * DH  # 1024 (q cols then k cols for this group)
VCOLS = HEADS_PER_CORE * DH  # 512
EPS = 1e-6

_NC_CACHE = {}


def build_bass():
    nc = bacc.Bacc("TRN2", target_bir_lowering=False, debug=False, num_devices=8)

    xT = nc.dram_tensor("xT", [D, S], F32, kind="ExternalInput")
    wqk = nc.dram_tensor("wqk", [D, QKCOLS], F32, kind="ExternalInput")
    wv = nc.dram_tensor("wv", [D, VCOLS], F32, kind="ExternalInput")
    wo = nc.dram_tensor("wo", [VCOLS, D], F32, kind="ExternalInput")
    onesblk = nc.dram_tensor("onesblk", [128, 2], F32, kind="ExternalInput")
    wqkblk = nc.dram_tensor("wqkblk", [2, 128], F32, kind="ExternalInput")
    maskpfx = nc.dram_tensor("maskpfx", [4, 128, 512], F32, kind="ExternalInput")
    bqk = nc.dram_tensor("bqk", [QKCOLS, 1], F32, kind="ExternalInput")
    bv = nc.dram_tensor("bv", [1, VCOLS], F32, kind="ExternalInput")
    outT = nc.dram_tensor("outT", [D, S], F32, kind="ExternalOutput")

    with tile.TileContext(nc) as tc:
        with ExitStack() as octx:
            persist = octx.enter_context(tc.tile_pool(name="persist", bufs=1))
            qn = persist.tile([128, 4, S], F32)  # normalized Q^T, 2 heads/row-tile
            kn = persist.tile([128, 4, S], F32)  # normalized K^T
            v_sb = persist.tile([128, 16, 8 * 65], F32)  # [V_h | ones] per head
            ob_sb = persist.tile([128, 2], F32)
            wb_sb = persist.tile([2, 128], F32)
            bqk_sb = persist.tile([128, 8], F32)
            bv_sb = persist.tile([128, VCOLS], F32)
            eps_sb = persist.tile([2, 1], F32)
            nc.vector.memset(eps_sb, EPS)

            nc.sync.dma_start(out=ob_sb, in_=onesblk.ap())
            nc.sync.dma_start(out=wb_sb, in_=wqkblk.ap())
            nc.sync.dma_start(
                out=bqk_sb,
                in_=bqk.ap().rearrange("(mt p) one -> p (mt one)", p=128),
            )
            bv_ap = bv.ap()
            bv_bcast = bass.AP(
                tensor=bv_ap.tensor,
                offset=bv_ap.offset,
                ap=[[0, 128], bv_ap.ap[1]],
            )
            nc.sync.dma_start(out=bv_sb, in_=bv_bcast)
            nc.vector.memset(v_sb, 1.0)  # presets the per-head ones columns

            # ---------------- phase 1: projections + rmsnorm ----------------
            with ExitStack() as p1:
                w1 = p1.enter_context(tc.tile_pool(name="w1", bufs=1))
                xs = p1.enter_context(tc.tile_pool(name="xs", bufs=2))
                scr = p1.enter_context(tc.tile_pool(name="scr", bufs=3))
                pr_pool = p1.enter_context(
                    tc.tile_pool(name="prp", bufs=4, space="PSUM")
                )
                ss_pool = p1.enter_context(
                    tc.tile_pool(name="ssp", bufs=2, space="PSUM")
                )
                bc_pool = p1.enter_context(
                    tc.tile_pool(name="bcp", bufs=2, space="PSUM")
                )

                wqk_sb = w1.tile([128, 8, QKCOLS], F32)
                wv_sb = w1.tile([128, 8, VCOLS], F32)
                nc.sync.dma_start(
                    out=wqk_sb, in_=wqk.ap().rearrange("(kc p) m -> p kc m", p=128)
                )
                nc.sync.dma_start(
                    out=wv_sb, in_=wv.ap().rearrange("(kc p) m -> p kc m", p=128)
                )

                xT_r = xT.ap().rearrange("(kc p) s -> p kc s", p=128)
                for qc in range(4):
                    xt = xs.tile([128, 8, 512], F32, tag="xt")
                    nc.sync.dma_start(
                        out=xt, in_=xT_r[:, :, qc * 512 : (qc + 1) * 512]
                    )
                    # Q^T / K^T projection, rmsnorm fused on eviction
                    for mt in range(8):
                        pr = pr_pool.tile([128, 512], F32, tag="pr")
                        for kc in range(8):
                            nc.tensor.matmul(
                                pr,
                                wqk_sb[:, kc, mt * 128 : (mt + 1) * 128],
                                xt[:, kc, :],
                                start=(kc == 0),
                                stop=(kc == 7),
                            )
                        b_ap = bqk_sb[:, mt : mt + 1]
                        sq = scr.tile([128, 512], F32, tag="sq")
                        # sq = (pr + b)^2
                        nc.scalar.activation(sq, pr, AF.Square, bias=b_ap)
                        ss = ss_pool.tile([2, 512], F32, tag="ss")
                        nc.tensor.matmul(ss, ob_sb, sq)
                        srt = scr.tile([2, 512], F32, tag="srt")
                        # srt = sqrt(ss/64 + eps)
                        nc.scalar.activation(
                            srt, ss, AF.Sqrt, bias=eps_sb, scale=1.0 / 64.0
                        )
                        inv = scr.tile([2, 512], F32, tag="inv")
                        nc.vector.reciprocal(inv, srt)
                        bc = bc_pool.tile([128, 512], F32, tag="bc")
                        # per-head broadcast of inv_rms, pre-scaled by wq*wk
                        # (wqkblk rows carry the per-dim weight factors)
                        nc.tensor.matmul(bc, wb_sb, inv)
                        bc_sb = scr.tile([128, 512], F32, tag="bcs")
                        nc.vector.tensor_copy(bc_sb, bc)
                        dst = qn if mt < 4 else kn
                        hp = mt % 4
                        # dst = (pr + b) * bc
                        nc.vector.scalar_tensor_tensor(
                            dst[:, hp, qc * 512 : (qc + 1) * 512],
                            pr,
                            b_ap,
                            bc_sb,
                            op0=ALU.add,
                            op1=ALU.mult,
                        )
                    # V projection
                    for sl in range(4):
                        vp = pr_pool.tile([128, 512], F32, tag="pr")
                        for kc in range(8):
                            nc.tensor.matmul(
                                vp,
                                xt[:, kc, sl * 128 : (sl + 1) * 128],
                                wv_sb[:, kc, :],
                                start=(kc == 0),
                                stop=(kc == 7),
                            )
                        sc = qc * 4 + sl
                        dstv = v_sb[:, sc, :].rearrange(
                            "p (h c) -> p h c", h=8
                        )[:, :, 0:64]
                        srcv = vp.rearrange("p (h c) -> p h c", h=8)
                        bvv = bv_sb.rearrange("p (h c) -> p h c", h=8)
                        nc.vector.tensor_add(dstv, srcv, bvv)

            # ---------------- phase 2+3: attention + out projection ----------
            with ExitStack() as p2:
                w2 = p2.enter_context(tc.tile_pool(name="w2", bufs=1))
                pp = p2.enter_context(tc.tile_pool(name="pp", bufs=4))
                zs = p2.enter_context(tc.tile_pool(name="zs", bufs=2))
                osb = p2.enter_context(tc.tile_pool(name="osb", bufs=3))
                rrp = p2.enter_context(tc.tile_pool(name="rrp", bufs=2))
                sp_pool = p2.enter_context(
                    tc.tile_pool(name="spp", bufs=2, space="PSUM")
                )
                zp_pool = p2.enter_context(
                    tc.tile_pool(name="zpp", bufs=2, space="PSUM")
                )
                op_pool = p2.enter_context(
                    tc.tile_pool(name="opp", bufs=2, space="PSUM")
                )

                wo_sb = w2.tile([128, 4, D], F32)
                nc.sync.dma_start(
                    out=wo_sb, in_=wo.ap().rearrange("(zc p) m -> p zc m", p=128)
                )
                mp_sb = w2.tile([128, 4, 512], F32)
                nc.sync.dma_start(
                    out=mp_sb, in_=maskpfx.ap().rearrange("c p f -> p c f")
                )

                for qb in range(4):
                    z_sb = zs.tile([128, 4, 512], F32, tag="z")
                    qsl = slice(qb * 512, (qb + 1) * 512)
                    for hp in range(4):
                        z_ps = [
                            zp_pool.tile(
                                [128, 512], F32, tag="zp", name=f"zps{hl}"
                            )
                            for hl in range(2)
                        ]
                        nchunks = 4 * (qb + 1)
                        for g2 in range(2 * (qb + 1)):
                            for hl in range(2):
                                h = 2 * hp + hl
                                psl = slice(hl * 64, (hl + 1) * 64)
                                s_ps = sp_pool.tile([128, 1024], F32, tag="sp")
                                for cc in range(2):
                                    kci = 2 * g2 + cc
                                    nc.tensor.matmul(
                                        s_ps[:, cc * 512 : (cc + 1) * 512],
                                        kn[psl, hp, kci * 128 : (kci + 1) * 128],
                                        qn[psl, hp, qsl],
                                    )
                                p_sb = pp.tile([128, 1024], F32, tag="p")
                                nc.scalar.activation(p_sb, s_ps, AF.Exp)
                                for cc in range(2):
                                    kci = 2 * g2 + cc
                                    ci = kci - 4 * qb
                                    if ci >= 0:
                                        w = 128 * ci + 128
                                        nc.vector.tensor_mul(
                                            p_sb[:, cc * 512 : cc * 512 + w],
                                            p_sb[:, cc * 512 : cc * 512 + w],
                                            mp_sb[:, ci, 0:w],
                                        )
                                for cc in range(2):
                                    kci = 2 * g2 + cc
                                    nc.tensor.matmul(
                                        z_ps[hl][0:65, :],
                                        v_sb[:, kci, h * 65 : (h + 1) * 65],
                                        p_sb[:, cc * 512 : (cc + 1) * 512],
                                        start=(kci == 0),
                                        stop=(kci == nchunks - 1),
                                        skip_group_check=True,
                                    )
                        for hl in range(2):
                            rr = rrp.tile([1, 512], F32, tag="rr")
                            nc.vector.reciprocal(rr, z_ps[hl][64:65, :])
                            rb = rrp.tile([64, 512], F32, tag="rb")
                            nc.gpsimd.partition_broadcast(rb, rr)
                            nc.vector.tensor_mul(
                                z_sb[hl * 64 : (hl + 1) * 64, hp, :],
                                z_ps[hl][0:64, :],
                                rb,
                            )
                    # out projection for this q-block
                    for mt in range(8):
                        o_ps = op_pool.tile([128, 512], F32, tag="op")
                        for zc in range(4):
                            nc.tensor.matmul(
                                o_ps,
                                wo_sb[:, zc, mt * 128 : (mt + 1) * 128],
                                z_sb[:, zc, :],
                                start=(zc == 0),
                                stop=(zc == 3),
                            )
                        ot = osb.tile([128, 512], F32, tag="ot")
                        nc.vector.tensor_copy(ot, o_ps)
                        nc.sync.dma_start(
                            out=outT.ap()[mt * 128 : (mt + 1) * 128, qsl], in_=ot
                        )

    nc.compile()
    return nc


def get_nc():
    if "nc" not in _NC_CACHE:
        _NC_CACHE["nc"] = build_bass()
    return _NC_CACHE["nc"]


def kernel(x, mask, W_qkv, b_qkv, W_o, b_o, wq, wk, _run_kwargs=None):
    x = np.asarray(x, dtype=np.float32)
    mask = np.asarray(mask)
    W_qkv = np.asarray(W_qkv, dtype=np.float32)
    b_qkv = np.asarray(b_qkv, dtype=np.float32)
    W_o = np.asarray(W_o, dtype=np.float32)
    b_o = np.asarray(b_o, dtype=np.float32)
    wq = np.asarray(wq, dtype=np.float32)
    wk = np.asarray(wk, dtype=np.float32)

    B = x.shape[0]
    n_groups = 2

    onesblk = np.zeros((128, 2), np.float32)
    onesblk[0:64, 0] = 1.0
    onesblk[64:128, 1] = 1.0

    wqkw = (wq * wk).astype(np.float32)  # [64]
    wqkblk = np.zeros((2, 128), np.float32)
    wqkblk[0, 0:64] = wqkw
    wqkblk[1, 64:128] = wqkw

    # 0/1 prefix mask tiles for the 4 diagonal-chunk offsets, built from the
    # actual mask input (causal block structure is assumed for chunk skipping).
    maskpfx = np.zeros((4, 128, 512), np.float32)
    for ci in range(4):
        blocked = mask[0:512, 128 * ci : 128 * ci + 128]  # [q=512, k=128]
        maskpfx[ci] = np.where(blocked.T, 0.0, 1.0)  # [k=128, q=512]

    Wq = W_qkv[:, 0:D]
    Wk = W_qkv[:, D : 2 * D]
    Wv = W_qkv[:, 2 * D : 3 * D]
    bq = b_qkv[0:D]
    bk = b_qkv[D : 2 * D]
    bvv = b_qkv[2 * D : 3 * D]

    in_maps = []
    for core in range(8):
        b = core // n_groups
        g = core % n_groups
        csl = slice(g * 512, (g + 1) * 512)
        wqk_c = np.ascontiguousarray(
            np.concatenate([Wq[:, csl], Wk[:, csl]], axis=1)
        )
        bqk_c = np.ascontiguousarray(
            np.concatenate([bq[csl], bk[csl]])[:, None]
        )
        in_maps.append(
            {
                "xT": np.ascontiguousarray(x[b].T),
                "wqk": wqk_c,
                "wv": np.ascontiguousarray(Wv[:, csl]),
                "wo": np.ascontiguousarray(W_o[csl, :]),
                "onesblk": onesblk,
                "wqkblk": wqkblk,
                "maskpfx": maskpfx,
                "bqk": bqk_c,
                "bv": np.ascontiguousarray(bvv[csl][None, :]),
            }
        )

    nc = get_nc()
    res = run_bass_kernel_spmd(
        nc, in_maps, list(range(8)), **(_run_kwargs or {})
    )

    out = np.empty((B, S, D), np.float32)
    for b in range(B):
        acc = res.results[2 * b]["outT"] + res.results[2 * b + 1]["outT"]
        out[b] = acc.T + b_o[None, :]
    if _run_kwargs is not None:
        _RUN_INFO["last"] = res
    return out


_RUN_INFO = {}
